# revision 83
# baseline (speedup 1.0000x reference)
"""Trainium2 Bass kernel for nn_BiGruBNattMaxFocalNet.

Data-parallel over batch: B=32 -> 4 per core x 8 cores.
Per-core pipeline (all feature-on-partition "family P" layouts):
  1. encoder input projections (f32r matmuls)
  2. shared BiGRU scans over context (T=100, L=4) and options (T=50, L=20)
  3. ctx_key / query projections
  4. per-(b,k) Bahdanau energy: DVE outer-add + ACT tanh + PE w_e-reduce
  5. exp / row-col sums / normalized attention matmuls
  6. fc/fo features -> attention-GRU input projections
  7. attention BiGRU scans with running max
  8. logits + softmax over K
"""
import ml_dtypes
import numpy as np
from contextlib import ExitStack

import concourse.bass as bass
import concourse.tile as tile
from concourse import mybir, masks
from concourse.bass_utils import run_bass_kernel_spmd
from concourse.vector_clock import ScopedClock

f32 = mybir.dt.float32
f32r = mybir.dt.float32r
bf16 = mybir.dt.bfloat16
AF = mybir.ActivationFunctionType
ALU = mybir.AluOpType

H, H2, H3, E, LC, LO, KOPT = 128, 256, 384, 300, 100, 50, 5
NCORES = 8
B4 = 4            # batch per core
LCTX = B4         # ctx scan lanes
LOPT = B4 * KOPT  # option scan lanes (=20)
NCTX = LC * LCTX      # 400 ctx (t,b) cols
NOPT = LO * LOPT      # 1000 opt (t,l) cols
NFC = LC * LOPT       # 2000 fc cols
NFO = LO * LOPT       # 1000 fo cols

# dtype used for the tanh-energy tiles consumed by the w_e-reduce matmul
# (bf16 halves the DVE outer-add cost via 2x_1p and enables FWL on the PE)
S_DT = bf16


class TC(tile.TileContext):
    """TileContext with walrus-compatible tail drain (<=1 wait per inst)."""

    def _drain_and_barrier(self, tick_clock, wait_clock):
        nc = self.nc
        probe = nc.sync.nop(nofuse=True)
        wait_clock.add_sem_waits(
            probe.ins, ScopedClock({None: tick_clock.global_clock})
        )
        si = probe.ins.sync_info
        waits = list(si.on_wait or [])
        si.on_wait = []
        assert self.sems is not None
        by_name = {h.name: h for h in self.sems.allocated().values()}
        for w in waits:
            nc.sync.wait_ge(by_name[w.ant_name], w.wait_value)
        nc.sync.drain()
        nc.all_engine_barrier()
        popped = nc._tile_sem_poison_stack.pop()
        assert popped is self._sem_poison
        nc.clear_and_free_semaphores(list(self.sems.allocated().values()))
        nc.all_engine_barrier()


def split_multi_waits(nc, max_waits=1):
    """This walrus build rejects >1 sync-wait per instruction; hoist extras
    onto same-engine NOPs placed immediately before the offender."""
    cnt = 0
    for fn in nc.m.functions:
        for bb in fn.blocks:
            insts = list(bb.instructions)
            out = []
            changed = False
            for inst in insts:
                si = inst.sync_info
                waits = list(si.on_wait) if si is not None and si.on_wait else []
                if len(waits) > max_waits:
                    changed = True
                    for w in waits[:-max_waits]:
                        cnt += 1
                        nop = mybir.InstNoOp(name=f"wait-split-{cnt}")
                        nop.engine = inst.engine
                        nop.sync_info = mybir.SyncInfo(on_wait=[w], on_update=[])
                        out.append(nop)
                    inst.sync_info = mybir.SyncInfo(
                        on_wait=waits[-max_waits:],
                        on_update=list(si.on_update or []),
                    )
                out.append(inst)
            if changed:
                bb.instructions = out
    return cnt


def _enc_projection(nc, ppj, xt_tiles, wihT, bias3, gxrz, gxn, segs):
    """input projections: gxrz[:, g, seg] (bf16, g<2) / gxn[:, seg] (f32).

    xt_tiles: list of (tile, rows) K-chunks of x.T (f32r). wihT: list of
    K-chunk tiles [rows, 384] (f32r). bias3: [128, 3].
    """
    for (lo, n) in segs:
        for g in range(3):
            ps = ppj.tile([128, 512], f32, tag="proj", name="ps_proj")
            for kc, (xt, rows) in enumerate(xt_tiles):
                nc.tensor.matmul(
                    ps[:, 0:n],
                    wihT[kc][0:rows, g * 128:(g + 1) * 128],
                    xt[0:rows, lo:lo + n],
                    start=(kc == 0), stop=(kc == len(xt_tiles) - 1),
                )
            out = gxn[:, lo:lo + n] if g == 2 else gxrz[:, g, lo:lo + n]
            nc.vector.tensor_scalar(
                out, ps[:, 0:n], bias3[:, g:g + 1], None, ALU.add
            )


def _gru_scans(nc, ctx, tc, scans, ident16, pfx="", psum_bufs=1):
    """Interleaved bidirectional GRU scans, family-P layout.

    scans: list of dicts with keys T, L, gx ({'f','b'}: [128, 3, T*L]
    projections incl rz biases + bih_n), whhT ({'f','b'}: [128, 384] f32r),
    bhhn ({'f','b'}: [128, 1] = b_hh[n]), outs (None or {'f','b'} [128, T*L]
    tiles), hmax (None or {'f','b'} [128, L] running-max tiles, written at
    t=0).  All scans step in lockstep over t so their (independent) serial
    chains interleave in the engine queues and hide each other's latency.
    sub/mul of the h-update run on gpsimd to offload the DVE.
    """
    hp = ctx.enter_context(tc.tile_pool(name=f"h{pfx}", bufs=4))
    vp = ctx.enter_context(tc.tile_pool(name=f"v{pfx}", bufs=4))
    # rz gates accumulate onto a preloaded bank; n gate needs its own bank
    # (matmul start=True zeroes the whole 2KB zero-region)
    gp = ctx.enter_context(tc.tile_pool(name=f"g{pfx}", bufs=2, space="PSUM"))
    gn = ctx.enter_context(tc.tile_pool(name=f"n{pfx}", bufs=psum_bufs,
                                        space="PSUM"))

    for s in scans:
        s["h"] = {"f": None, "b": None}
    Tmax = max(s["T"] for s in scans)
    for t in range(Tmax):
        for si, s in enumerate(scans):
            T, L = s["T"], s["L"]
            if t >= T:
                continue
            for d in ("f", "b"):
                col = (t if d == "f" else T - 1 - t) * L
                GXR, GXN = s["gx"][d]
                BN = s["bhhn"][d]
                rz = vp.tile([128, 2, L], f32, tag=f"rz{si}{d}", name=f"rz{si}{d}")
                sn = vp.tile([128, L], f32, tag=f"sn{si}{d}", name=f"sn{si}{d}")
                if t == 0:
                    # h==0: recurrent matmuls vanish; gates_rz = gx_rz,
                    # gates_n = bhh_n
                    nc.scalar.activation(rz[:], GXR[:, 0:2, col:col + L],
                                         AF.Sigmoid)
                    nc.vector.scalar_tensor_tensor(
                        sn[:], rz[:, 0, :], BN[:, 0:1],
                        GXN[:, col:col + L], ALU.mult, ALU.add)
                else:
                    grz = gp.tile([128, 2, L], f32, tag=f"g{d}",
                                  name=f"grz{si}{d}")
                    gnn = gn.tile([128, L], f32, tag=f"n{d}",
                                  name=f"gn{si}{d}")
                    hr = (s["h"][d][:, 0:L] if s.get("bf")
                          else s["h"][d][:, 0:L].bitcast(f32r))
                    # inject gx_rz into the accumulation group via an
                    # identity matmul (same engine/group: race-free), then
                    # accumulate the recurrent products on top
                    nc.tensor.matmul(grz[:], ident16[:, :],
                                     GXR[:, 0:2, col:col + L],
                                     start=True, stop=False)
                    for g in range(2):
                        nc.tensor.matmul(
                            grz[:, g, :],
                            s["whhT"][d][:, g * 128:(g + 1) * 128], hr,
                            start=False, stop=(g == 1),
                        )
                    nc.tensor.matmul(gnn[:], s["whhT"][d][:, 256:384], hr,
                                     start=True, stop=True)
                    nc.scalar.activation(rz[:], grz[:], AF.Sigmoid)
                    tn = vp.tile([128, L], f32, tag=f"tn{si}{d}",
                                 name=f"tn{si}{d}")
                    nc.vector.scalar_tensor_tensor(
                        tn[:], gnn[:], BN[:, 0:1], rz[:, 0, :],
                        ALU.add, ALU.mult)
                    if s.get("bf"):
                        nc.gpsimd.tensor_add(sn[:], tn[:],
                                             GXN[:, col:col + L])
                    else:
                        nc.vector.tensor_add(sn[:], tn[:],
                                             GXN[:, col:col + L])
                n_t = vp.tile([128, L], f32, tag=f"n{si}{d}", name=f"n{si}{d}")
                nc.scalar.activation(n_t[:], sn[:], AF.Tanh)
                if s["outs"] is not None:
                    hn = s["outs"][d][:, col:col + L]
                else:
                    hn = hp.tile([128, L], f32, tag=f"h{si}{d}",
                                 name=f"h{si}{d}")[:, 0:L]
                # hn = (1-z)*n + z*h; omz (and zh for f32 scans) off-chain
                # on gpsimd -- gpsimd never touches bf16 (software converts)
                hw = hn if s.get("bf") else hn.bitcast(f32r)
                omz = vp.tile([128, L], f32, tag=f"om{si}{d}", name=f"om{si}{d}")
                nc.gpsimd.tensor_scalar(omz[:], rz[:, 1, :], -1.0, 1.0,
                                        ALU.mult, ALU.add)
                if t == 0:
                    nc.vector.tensor_mul(hw, omz[:], n_t[:])
                else:
                    zh = vp.tile([128, L], f32, tag=f"zh{si}{d}",
                                 name=f"zh{si}{d}")
                    if s.get("bf"):
                        nc.vector.tensor_mul(zh[:], rz[:, 1, :],
                                             s["h"][d][:, 0:L])
                    else:
                        nc.gpsimd.tensor_mul(zh[:], rz[:, 1, :],
                                             s["h"][d][:, 0:L])
                    mm_ = vp.tile([128, L], f32, tag=f"mm{si}{d}",
                                  name=f"mm{si}{d}")
                    nc.vector.tensor_mul(mm_[:], omz[:], n_t[:])
                    nc.vector.tensor_add(hw, mm_[:], zh[:])
                s["h"][d] = hn


DEBUG = False
_BUILT = {}


def _build():
    nc = bass.Bass("TRN2", target_bir_lowering=False, debug=False)
    dram = {}

    def din(name, shape, dt=f32):
        dram[name] = nc.dram_tensor(name, list(shape), dt, kind="ExternalInput").ap()
        return dram[name]

    # sharded activations (host pre-transposed)
    din("ctxT", [E, NCTX])      # (e, (t, b))
    din("optT", [E, NOPT])      # (e, (t, k*4+b))
    # encoder weights
    for d in ("f", "b"):
        din(f"wihT_{d}", [E, H3])
        din(f"whhT_{d}", [H, H3], bf16)
        din(f"bias3_{d}", [H, 3])     # [:,0:2]=bih+bhh rz, [:,2]=bih_n
        din(f"bhhn_{d}", [H, 1])
        din(f"awihT_{d}", [8 * H, H3], bf16)
        din(f"awhhT_{d}", [H, H3], bf16)
        din(f"abias3_{d}", [H, 3])
        din(f"abhhn_{d}", [H, 1])
    din("wkT", [H2, H2], bf16)
    din("wqT", [H2, H2], bf16)
    din("wemat", [H2, H2])
    din("vvec", [H2, 1])
    din("wsimT", [4 * H, 1], bf16)
    out_ap = nc.dram_tensor("out", [B4, KOPT], f32, kind="ExternalOutput").ap()
    dbg = {}
    if DEBUG:
        for nm, shape in [
            ("d_ctxf", [H, NCTX]), ("d_ctxb", [H, NCTX]),
            ("d_optf", [H, NOPT]), ("d_optb", [H, NOPT]),
            ("d_scores", [LO, LC]), ("d_E", [LO, LC]),
            ("d_ck", [H, 2, NCTX]), ("d_q", [H, 2, NOPT]), ("d_we", [H, 2]),
            ("d_S0", [H, 10, LC]),
            ("d_acx", [H, 2, NFC]), ("d_aop", [H, 2, NFO]),
            ("d_hcf", [H, LOPT]), ("d_hcb", [H, LOPT]),
            ("d_hof", [H, LOPT]), ("d_hob", [H, LOPT]),
            ("d_logits", [1, LOPT]),
        ]:
            dbg[nm] = nc.dram_tensor(nm, shape, f32, kind="ExternalOutput").ap()

    with TC(nc) as tc, ExitStack() as ctx:
        pw = ctx.enter_context(tc.tile_pool(name="pw", bufs=1))
        pm = ctx.enter_context(tc.tile_pool(name="pm", bufs=1))
        pj_ctx = ExitStack()
        ppj = pj_ctx.enter_context(tc.tile_pool(name="ppj", bufs=2, space="PSUM"))

        # ---- load weights ----
        W = {}
        for d in ("f", "b"):
            W[f"wihT_{d}"] = [pw.tile([128, H3], f32r, name=f"wih{d}{kc}")
                              for kc in range(3)]
            for kc in range(3):
                rows = min(128, E - kc * 128)
                nc.gpsimd.dma_start(W[f"wihT_{d}"][kc][0:rows, :],
                                    dram[f"wihT_{d}"][kc * 128:kc * 128 + rows, :])
            W[f"awihT_{d}"] = [pw.tile([128, H3], bf16, name=f"awih{d}{kc}")
                               for kc in range(8)]
            for kc in range(8):
                nc.gpsimd.dma_start(W[f"awihT_{d}"][kc][:],
                                    dram[f"awihT_{d}"][kc * 128:(kc + 1) * 128, :])
            W[f"whhT_{d}"] = pw.tile([128, H3], bf16, name=f"whhT_{d}")
            nc.gpsimd.dma_start(W[f"whhT_{d}"][:], dram[f"whhT_{d}"][:])
            W[f"awhhT_{d}"] = pw.tile([128, H3], bf16, name=f"awhhT_{d}")
            nc.gpsimd.dma_start(W[f"awhhT_{d}"][:], dram[f"awhhT_{d}"][:])
            for nm in (f"bias3_{d}", f"abias3_{d}"):
                W[nm] = pw.tile([128, 3], f32, name=nm)
                nc.sync.dma_start(W[nm][:], dram[nm][:])
            for nm in (f"bhhn_{d}", f"abhhn_{d}"):
                W[nm] = pw.tile([128, 1], f32, name=nm)
                nc.sync.dma_start(W[nm][:], dram[nm][:])
        for nm in ("wkT", "wqT"):
            W[nm] = [pw.tile([128, H2], bf16, name=f"{nm}{kc}") for kc in range(2)]
            for kc in range(2):
                nc.gpsimd.dma_start(W[nm][kc][:], dram[nm][kc * 128:(kc + 1) * 128, :])
        W["wsimT"] = [pw.tile([128, 1], bf16, name=f"wsimT{kc}") for kc in range(4)]
        for kc in range(4):
            nc.gpsimd.dma_start(W["wsimT"][kc][:], dram["wsimT"][kc * 128:(kc + 1) * 128, :])
        ident = pw.tile([128, 128], f32, name="ident")
        masks.make_identity(nc, ident[:])
        ident16 = pw.tile([128, 128], bf16, name="ident16")
        nc.vector.tensor_copy(ident16[:], ident[:])

        # w_e = We.T @ v, one [128, 2] tile (col per hc); reduce-matmul lhsT
        # slices are bitcast f32r views.
        wemat = [pw.tile([128, H2], f32, name=f"wemat{kc}") for kc in range(2)]
        for kc in range(2):
            nc.sync.dma_start(wemat[kc][:], dram["wemat"][kc * 128:(kc + 1) * 128, :])
        vtile = pw.tile([128, 2], f32, name="vtile")
        nc.sync.dma_start(vtile[:], dram["vvec"][:].rearrange("(a p) o -> p (a o)", a=2))
        we_ps = ppj.tile([128, 512], f32, tag="proj", name="we_ps")
        for hc in range(2):
            for jc in range(2):
                nc.tensor.matmul(we_ps[:, hc:hc + 1],
                                 wemat[jc][:, hc * 128:(hc + 1) * 128],
                                 vtile[:, jc:jc + 1],
                                 start=(jc == 0), stop=(jc == 1))
        we = pw.tile([128, 2], bf16, name="we")
        nc.vector.tensor_copy(we[:], we_ps[:, 0:2])
        weRep = [we[:, hc:hc + 1] for hc in range(2)]

        # ---- load activations (f32r) ----
        penc_ctx = ExitStack()
        penc = penc_ctx.enter_context(tc.tile_pool(name="penc", bufs=1))
        ctxT = [penc.tile([128, NCTX], f32r, name=f"ctxT{kc}") for kc in range(3)]
        optT = [penc.tile([128, NOPT], f32r, name=f"optT{kc}") for kc in range(3)]
        for kc in range(3):
            rows = min(128, E - kc * 128)
            nc.gpsimd.dma_start(ctxT[kc][0:rows, :], dram["ctxT"][kc * 128:kc * 128 + rows, :])
            nc.gpsimd.dma_start(optT[kc][0:rows, :], dram["optT"][kc * 128:kc * 128 + rows, :])
        xt_ctx = [(ctxT[0], 128), (ctxT[1], 128), (ctxT[2], 44)]
        xt_opt = [(optT[0], 128), (optT[1], 128), (optT[2], 44)]

        # ---- encoder gx ----
        gx1c = {}
        gx1o = {}
        for d in ("f", "b"):
            gx1c[d] = (penc.tile([128, 2, NCTX], bf16, name=f"gx1cr{d}"),
                       penc.tile([128, NCTX], f32, name=f"gx1cn{d}"))
            gx1o[d] = (penc.tile([128, 2, NOPT], bf16, name=f"gx1or{d}"),
                       penc.tile([128, NOPT], f32, name=f"gx1on{d}"))
            _enc_projection(nc, ppj, xt_ctx, W[f"wihT_{d}"], W[f"bias3_{d}"],
                            gx1c[d][0], gx1c[d][1], [(0, 400)])
            _enc_projection(nc, ppj, xt_opt, W[f"wihT_{d}"], W[f"bias3_{d}"],
                            gx1o[d][0], gx1o[d][1], [(0, 500), (500, 500)])

        # ---- encoder scans (ctx & opt interleaved: 4 independent chains) ----
        ctx_o = {d: pm.tile([128, NCTX], bf16, name=f"ctxo{d}") for d in ("f", "b")}
        opt_o = {d: pm.tile([128, NOPT], bf16, name=f"opto{d}") for d in ("f", "b")}
        enc_whh = {d: W[f"whhT_{d}"] for d in ("f", "b")}
        enc_bhn = {d: W[f"bhhn_{d}"] for d in ("f", "b")}
        with ExitStack() as sctx:
            _gru_scans(nc, sctx, tc, [
                dict(T=LC, L=LCTX, gx={d: gx1c[d] for d in ("f", "b")},
                     whhT=enc_whh, bhhn=enc_bhn, outs=ctx_o, hmax=None,
                     bf=True),
                dict(T=LO, L=LOPT, gx={d: gx1o[d] for d in ("f", "b")},
                     whhT=enc_whh, bhhn=enc_bhn, outs=opt_o, hmax=None,
                     bf=True),
            ], ident16, pfx="se")
        if DEBUG:
            nc.sync.dma_start(dbg["d_ctxf"][:], ctx_o["f"][:])
            nc.sync.dma_start(dbg["d_ctxb"][:], ctx_o["b"][:])
            nc.sync.dma_start(dbg["d_optf"][:], opt_o["f"][:])
            nc.sync.dma_start(dbg["d_optb"][:], opt_o["b"][:])
        penc_ctx.close()

        # ---- ck / q projections (family P; bf16 outputs for the energy
        # phase: ck stored b-major so SA adds read packed last dims) ----
        ck = pm.tile([128, 2, B4, LC], bf16, name="ck")
        qq = pm.tile([128, 2, NOPT], bf16, name="qq")
        for mc in range(2):
            ps = ppj.tile([128, 512], f32, tag="proj", name="ck_ps")
            for kc, d in enumerate(("f", "b")):
                nc.tensor.matmul(ps[:, 0:NCTX],
                                 W["wkT"][kc][:, mc * 128:(mc + 1) * 128],
                                 ctx_o[d][:],
                                 start=(kc == 0), stop=(kc == 1))
            # psum cols are (c, b); scatter to ck[mc, b, c]
            ck_out = bass.AP(tensor=ck.tensor, offset=ck.offset + mc * B4 * LC,
                             ap=[list(ck.ap[0]), [1, LC], [LC, B4]])
            nc.vector.tensor_copy(ck_out, ps[:, 0:NCTX]
                                  .rearrange("p (c b) -> p c b", b=B4))
            for s in range(2):
                ps2 = ppj.tile([128, 512], f32, tag="proj", name="q_ps")
                for kc, d in enumerate(("f", "b")):
                    nc.tensor.matmul(ps2[:, 0:500],
                                     W["wqT"][kc][:, mc * 128:(mc + 1) * 128],
                                     opt_o[d][:, s * 500:(s + 1) * 500],
                                     start=(kc == 0), stop=(kc == 1))
                # psum cols are (i, l); scatter to qq[mc, l, i] (l-major so
                # per-lane i-slices are contiguous for the 2x_1p SA adds)
                q_out = bass.AP(
                    tensor=qq.tensor,
                    offset=qq.offset + mc * NOPT + s * 25,
                    ap=[list(qq.ap[0]), [1, 25], [LO, LOPT]])
                nc.vector.tensor_copy(
                    q_out, ps2[:, 0:500].rearrange("p (i l) -> p i l", l=LOPT))

        if DEBUG:
            nc.sync.dma_start(dbg["d_ck"][:], ck[:])
            nc.sync.dma_start(dbg["d_q"][:], qq[:])
            nc.sync.dma_start(dbg["d_we"][:], we[:])
        # ---- energy + attention per (b, k) pair ----
        # pair lane l = k*4 + b; opt cols (i, l) = i*20+l; ctx cols (c, b) = c*4+b
        acx = pm.tile([128, 2, LC, LOPT], bf16, name="acx")  # attn_ctx (c, l)
        aop = pm.tile([128, 2, NFO], bf16, name="aop")       # attn_opt (i, l)
        NB = 10  # i-block size
        pj_ctx.close()
        with ExitStack() as ectx:
            pe_s = ectx.enter_context(tc.tile_pool(name="pe_s", bufs=6))
            pe_m = ectx.enter_context(tc.tile_pool(name="pe_m", bufs=6))
            pp_sc = ectx.enter_context(tc.tile_pool(name="pp_sc", bufs=2, space="PSUM"))
            pp_at = ectx.enter_context(tc.tile_pool(name="pp_at", bufs=2, space="PSUM"))
            pp_tr = ectx.enter_context(tc.tile_pool(name="pp_tr", bufs=2, space="PSUM"))
            pdram = ectx.enter_context(tc.tile_pool(name="pdram", bufs=5,
                                                    space="DRAM"))
            # hoist ctx_B transposes out of the lane loop (depend on b, d only)
            pcb = ectx.enter_context(tc.tile_pool(name="pcb", bufs=1))
            ctxB_all = {}
            for d in ("f", "b"):
                for b_ in range(B4):
                    ctxB_ps = pp_tr.tile([LC, 128], bf16, tag="tr",
                                         name="ctxB_ps")
                    cin_view = bass.AP(tensor=ctx_o[d].tensor,
                                       offset=ctx_o[d].offset + b_,
                                       ap=[list(ctx_o[d].ap[0]), [B4, LC]])
                    nc.tensor.transpose(ctxB_ps[:], cin_view, ident16[:, :])
                    cb = pcb.tile([LC, 128], bf16, name=f"ctxB{d}{b_}")
                    nc.vector.tensor_copy(cb[:], ctxB_ps[:])
                    ctxB_all[(d, b_)] = cb
            # hoist the ck broadcast-over-i (c-major, per (hc, b)): one
            # [128, LC, NB] bf16 tile each, so SA adds read packed bf16
            ckB = pcb.tile([128, 2, B4, LC, NB], bf16, name="ckB")
            for hc in range(2):
                for b_ in range(B4):
                    ck_src = bass.AP(
                        tensor=ck.tensor,
                        offset=ck.offset + (hc * B4 + b_) * LC,
                        ap=[list(ck.ap[0]), [1, LC], [0, NB]])
                    nc.vector.tensor_copy(ckB[:, hc, b_, :, :], ck_src)
            ones32 = pw.tile([128, 128], f32, name="ones32")
            nc.vector.memset(ones32[:], 1.0)
            ones16 = pw.tile([128, 128], bf16, name="ones16")
            nc.vector.tensor_copy(ones16[:], ones32[:])
            for l in range(LOPT):
                kk, bb = divmod(l, B4)
                # scores assembled TRANSPOSED: EtT[c, i] (c-major blocks)
                EtT = pe_m.tile([LC, LO], f32, tag="E", name="EtT")
                dsc = pdram.tile([1, LO * LC], f32, tag="dsc", name="dsc")
                for ib in range(LO // NB):
                    ssc = pp_sc.tile([1, 2, 512], f32, tag="ssc", name="ssc")
                    # SA[h, (hc, c, i)] = q[h, hc, i] + ck[h, hc, c]: both hc
                    # halves in one packed-bf16 add + one tanh (halves the
                    # per-op init overhead on the bottleneck engines)
                    SA = pe_s.tile([128, 2, LC, NB], bf16, tag="SA", name="SA")
                    S = pe_s.tile([128, 2, LC, NB], S_DT, tag="S", name="S")
                    q_view = bass.AP(
                        tensor=qq.tensor,
                        offset=qq.offset + l * LO + ib * NB,
                        ap=[list(qq.ap[0]), [NOPT, 2], [0, LC], [1, NB]])
                    ck_view = bass.AP(
                        tensor=ckB.tensor,
                        offset=ckB.offset + bb * LC * NB,
                        ap=[list(ckB.ap[0]), [B4 * LC * NB, 2], [NB, LC],
                            [1, NB]])
                    nc.vector.tensor_add(SA[:], q_view, ck_view)
                    nc.scalar.activation(
                        S[:].rearrange("p a b c -> p a (b c)"),
                        SA[:].rearrange("p a b c -> p a (b c)"), AF.Tanh)
                    for hc in range(2):
                        Sf = S[:, hc, :, :].rearrange("p a b -> p (a b)")
                        for j in range(2):
                            nc.tensor.matmul(
                                ssc[:, j, 0:500],
                                weRep[hc],
                                Sf[:, j * 500:(j + 1) * 500],
                                start=(hc == 0), stop=(hc == 1))
                    # psum row [1, 2, 500] -> sbuf -> dram (flat)
                    srow = pe_m.tile([1, 2, 500], f32, tag="srow", name="srow")
                    if ib % 2 == 1:
                        nc.scalar.copy(srow[:], ssc[:, :, 0:500])
                    else:
                        nc.vector.tensor_copy(srow[:], ssc[:, :, 0:500])
                    nc.sync.dma_start(
                        dsc[:, ib * NB * LC:(ib + 1) * NB * LC], srow[:])
                # gather dram rows into EtT[c, i]:
                # flat n = ib*1000 + j*500 + c_local*10 + i_local,
                # c = j*50 + c_local, i = ib*10 + i_local
                nc.sync.dma_start(
                    EtT[:],
                    bass.AP(tensor=dsc.tensor, offset=dsc.offset,
                            ap=[[500, 2], [10, 50], [1000, 5], [1, 10]]))
                EtT16 = pe_m.tile([LC, LO], bf16, tag="E16", name="EtT16")
                nc.scalar.activation(EtT16[:], EtT[:], AF.Exp)
                # softmax-over-c weights (attn_opt): smcT[c,i] = E/colsum_c,
                # denominators via ones-matmul (replicated on partitions)
                sumc_ps = pp_at.tile([128, 512], f32, tag="at", name="sumc_ps")
                nc.tensor.matmul(sumc_ps[:, 0:LO], ones16[0:LC, :],
                                 EtT16[:], start=True, stop=True)
                recipB = pe_m.tile([128, LO], f32, tag="rcb", name="recipB")
                nc.vector.reciprocal(recipB[:], sumc_ps[:, 0:LO])
                smcT = pe_m.tile([LC, LO], bf16, tag="smcTs", name="smcT")
                nc.vector.tensor_mul(smcT[:], EtT16[:], recipB[0:LC, :])
                # transpose E -> [i, c] for attn_ctx
                EtS_ps = pp_tr.tile([LC, 128], bf16, tag="tr", name="EtS_ps")
                nc.tensor.transpose(EtS_ps[0:LO, 0:LC], EtT16[:],
                                    ident16[0:LC, 0:LC])
                EtS = pe_m.tile([LO, LC], bf16, tag="EtS_s", name="EtS")
                nc.vector.tensor_copy(EtS[:], EtS_ps[0:LO, 0:LC])
                si_ps = pp_at.tile([128, 512], f32, tag="at", name="si_ps")
                nc.tensor.matmul(si_ps[:, 0:LC], ones16[0:LO, :],
                                 EtS[:], start=True, stop=True)
                rsi_b = pe_m.tile([128, LC], f32, tag="rsib", name="rsi_b")
                nc.vector.reciprocal(rsi_b[:], si_ps[:, 0:LC])
                # attn_ctx = (opt_B.T @ E) * rsi_b ; attn_opt = ctx_B.T @ smcT
                for hc, d in enumerate(("f", "b")):
                    optB_ps = pp_tr.tile([LC, 128], bf16, tag="tr",
                                         name="optB_ps")[0:LO, :]
                    in_view = bass.AP(tensor=opt_o[d].tensor,
                                      offset=opt_o[d].offset + l,
                                      ap=[list(opt_o[d].ap[0]), [LOPT, LO]])
                    nc.tensor.transpose(optB_ps[:], in_view, ident16[:, :])
                    optB = pe_m.tile([LO, 128], bf16, tag="optB_s", name="optB")
                    nc.vector.tensor_copy(optB[:], optB_ps[:])
                    acx_ps = pp_at.tile([128, 512], f32, tag="at", name="acx_ps")
                    nc.tensor.matmul(acx_ps[:, 0:LC], optB[:], EtS[:],
                                     start=True, stop=True)
                    acx_view = bass.AP(
                        tensor=acx.tensor,
                        offset=acx.offset + (hc * LC * LOPT + l),
                        ap=[list(acx.ap[0]), [LOPT, LC]])
                    nc.vector.tensor_mul(acx_view,
                                         acx_ps[:, 0:LC], rsi_b[:])

                    aop_ps = pp_at.tile([128, 512], f32, tag="at", name="aop_ps")
                    nc.tensor.matmul(aop_ps[:, 0:LO], ctxB_all[(d, bb)][:],
                                     smcT[:], start=True, stop=True)
                    aop_view = bass.AP(
                        tensor=aop.tensor,
                        offset=aop.offset + (hc * NFO + l),
                        ap=[list(aop.ap[0]), [LOPT, LO]])
                    nc.vector.tensor_copy(aop_view, aop_ps[:, 0:LO])
        if DEBUG:
            nc.sync.dma_start(dbg["d_acx"][:], acx[:].rearrange("p a b c -> p a (b c)"))
            nc.sync.dma_start(dbg["d_aop"][:], aop[:])

        # ---- attention-GRU: gx2 projections for both shifts, then the two
        # scans interleaved (4 independent chains) ----
        hmx = {}
        gx2s = {}
        with ExitStack() as actx:
            pg2 = actx.enter_context(tc.tile_pool(name="pg2", bufs=1))
            for shift in ("fc", "fo"):
                T2 = LC if shift == "fc" else LO
                N2 = T2 * LOPT
                pfs_ctx = ExitStack()
                pfs = pfs_ctx.enter_context(tc.tile_pool(name=f"pfs{shift}",
                                                         bufs=1))
                pp2 = pfs_ctx.enter_context(tc.tile_pool(name=f"pp2{shift}",
                                                         bufs=2, space="PSUM"))
                gx2 = {d: (pg2.tile([128, 2, N2], bf16,
                                    name=f"gx2r{shift}{d}"),
                           pg2.tile([128, N2], f32, name=f"gx2n{shift}{d}"))
                       for d in ("f", "b")}
                gx2s[shift] = gx2
                # feature chunk sources (family P, col = (t, l)); first build
                # per-slice f32r buffers, then accumulate the projection.
                if shift == "fc":
                    base = {d: ctx_o[d] for d in ("f", "b")}
                    att = acx[:].rearrange("p a b c -> p a (b c)")
                    bview = {d: bass.AP(
                        tensor=base[d].tensor, offset=base[d].offset,
                        ap=[list(base[d].ap[0]), [B4, LC], [0, KOPT], [1, B4]])
                        for d in ("f", "b")}
                else:
                    base = {d: opt_o[d] for d in ("f", "b")}
                    att = aop[:]
                    bview = {d: base[d][:].rearrange("p (t l) -> p t l", l=LOPT)
                             for d in ("f", "b")}
                nslices = N2 // 500
                slice_order = [nslices - 1] + list(range(nslices - 1))
                for s in slice_order:
                    # 8 feature chunks for this 500-col slice, f32r
                    fsl = []
                    for hc, d in enumerate(("f", "b")):
                        if shift == "fc":
                            fR = pfs.tile([128, 500], bf16, tag=f"fb{hc}", name=f"fb{hc}")
                            nc.vector.tensor_copy(
                                fR[:].rearrange("p (a b) -> p a b", b=LOPT)
                                .rearrange("p a (k c) -> p a k c", k=KOPT),
                                bview[d][:, s * 25:(s + 1) * 25, :, :])
                        else:
                            fR = base[d][:, s * 500:(s + 1) * 500]
                        fsl.append(fR)
                    for hc in range(2):
                        fsl.append(att[:, hc, s * 500:(s + 1) * 500])
                    for hc, d in enumerate(("f", "b")):
                        pR = pfs.tile([128, 500], bf16, tag=f"fp{hc}", name=f"fp{hc}")
                        nc.vector.tensor_mul(
                            pR[:].rearrange("p (a b) -> p a b", b=LOPT)
                            if shift == "fc" else pR[:],
                            bview[d][:, s * 25:(s + 1) * 25, :, :]
                            if shift == "fc" else base[d][:, s * 500:(s + 1) * 500],
                            att[:, hc, s * 500:(s + 1) * 500].rearrange(
                                "p (a b) -> p a b", b=LOPT)
                            if shift == "fc" else att[:, hc, s * 500:(s + 1) * 500])
                        fsl.append(pR)
                    for hc, d in enumerate(("f", "b")):
                        dR = pfs.tile([128, 500], bf16, tag=f"fd{hc}", name=f"fd{hc}")
                        nc.vector.tensor_sub(
                            dR[:].rearrange("p (a b) -> p a b", b=LOPT)
                            if shift == "fc" else dR[:],
                            bview[d][:, s * 25:(s + 1) * 25, :, :]
                            if shift == "fc" else base[d][:, s * 500:(s + 1) * 500],
                            att[:, hc, s * 500:(s + 1) * 500].rearrange(
                                "p (a b) -> p a b", b=LOPT)
                            if shift == "fc" else att[:, hc, s * 500:(s + 1) * 500])
                        fsl.append(dR)
                    for d in ("f", "b"):
                        for g in range(3):
                            ps = pp2.tile([128, 512], f32, tag="p2", name="gx2_ps")
                            for fch in range(8):
                                nc.tensor.matmul(
                                    ps[:, 0:500],
                                    W[f"awihT_{d}"][fch][:, g * 128:(g + 1) * 128],
                                    fsl[fch] if isinstance(fsl[fch], bass.AP)
                                    else fsl[fch][:],
                                    start=(fch == 0), stop=(fch == 7))
                            g2out = (gx2[d][1][:, s * 500:(s + 1) * 500]
                                     if g == 2 else
                                     gx2[d][0][:, g, s * 500:(s + 1) * 500])
                            nc.vector.tensor_scalar(
                                g2out, ps[:, 0:500],
                                W[f"abias3_{d}"][:, g:g + 1], None, ALU.add)
                pfs_ctx.close()
                hmx[shift] = {d: pm.tile([128, LOPT], bf16,
                                         name=f"hmx{shift}{d}")
                              for d in ("f", "b")}
            att_whh = {d: W[f"awhhT_{d}"] for d in ("f", "b")}
            att_bhn = {d: W[f"abhhn_{d}"] for d in ("f", "b")}
            pao = actx.enter_context(tc.tile_pool(name="pao", bufs=1))
            aouts = {shift: {d: pao.tile([128, (LC if shift == "fc" else LO)
                                          * LOPT], bf16,
                                         name=f"ao{shift}{d}")
                             for d in ("f", "b")}
                     for shift in ("fc", "fo")}
            with ExitStack() as sctx:
                _gru_scans(nc, sctx, tc, [
                    dict(T=LC, L=LOPT, gx=gx2s["fc"], whhT=att_whh,
                         bhhn=att_bhn, outs=aouts["fc"], hmax=None, bf=True),
                    dict(T=LO, L=LOPT, gx=gx2s["fo"], whhT=att_whh,
                         bhhn=att_bhn, outs=aouts["fo"], hmax=None, bf=True),
                ], ident16, pfx="sa", psum_bufs=2)
            # max over t in one strided reduce per (shift, dir)
            for shift, T2 in (("fc", LC), ("fo", LO)):
                for d in ("f", "b"):
                    ov = bass.AP(tensor=aouts[shift][d].tensor,
                                 offset=aouts[shift][d].offset,
                                 ap=[list(aouts[shift][d].ap[0]),
                                     [1, LOPT], [LOPT, T2]])
                    hv = bass.AP(tensor=hmx[shift][d].tensor,
                                 offset=hmx[shift][d].offset,
                                 ap=[list(hmx[shift][d].ap[0]),
                                     [1, LOPT], [0, 1]])
                    nc.vector.tensor_reduce(hv, ov,
                                            mybir.AxisListType.X, ALU.max)
        if DEBUG:
            nc.sync.dma_start(dbg["d_hcf"][:], hmx["fc"]["f"][:])
            nc.sync.dma_start(dbg["d_hcb"][:], hmx["fc"]["b"][:])
            nc.sync.dma_start(dbg["d_hof"][:], hmx["fo"]["f"][:])
            nc.sync.dma_start(dbg["d_hob"][:], hmx["fo"]["b"][:])

        # ---- logits + softmax over K ----
        with ExitStack() as lctx:
            plg = lctx.enter_context(tc.tile_pool(name="plg", bufs=1))
            pplg = lctx.enter_context(tc.tile_pool(name="pplg", bufs=1, space="PSUM"))
            featR = [hmx[shift][d][:]
                     for shift in ("fc", "fo") for d in ("f", "b")]
            lg_ps = pplg.tile([1, 512], f32, name="lg_ps")
            for i, fR in enumerate(featR):
                nc.tensor.matmul(lg_ps[:, 0:LOPT], W["wsimT"][i], fR,
                                 start=(i == 0), stop=(i == 3))
            lg_row = plg.tile([1, LOPT], f32, name="lg_row")
            nc.vector.tensor_copy(lg_row[:], lg_ps[:, 0:LOPT])
            if DEBUG:
                nc.sync.dma_start(dbg["d_logits"][:], lg_row[:])
            pldram = lctx.enter_context(tc.tile_pool(name="pldram", bufs=1,
                                                     space="DRAM"))
            dlg = pldram.tile([1, LOPT], f32, name="dlg")
            nc.sync.dma_start(dlg[:], lg_row[:])
            lg = plg.tile([B4, KOPT], f32, name="lg")
            nc.sync.dma_start(
                lg[:], bass.AP(tensor=dlg.tensor, offset=dlg.offset,
                               ap=[[1, B4], [B4, KOPT]]))
            mx = plg.tile([B4, 1], f32, name="mx")
            nc.vector.tensor_reduce(mx[:], lg[:], mybir.AxisListType.X, ALU.max,
                                    negate=True)
            ex = plg.tile([B4, KOPT], f32, name="ex")
            sm = plg.tile([B4, 1], f32, name="sm")
            nc.scalar.activation(ex[:], lg[:], AF.Exp, bias=mx[:], accum_out=sm[:])
            nc.vector.reciprocal(sm[:], sm[:])
            prob = plg.tile([B4, KOPT], f32, name="prob")
            nc.vector.tensor_scalar(prob[:], ex[:], sm[:], None, ALU.mult)
            nc.sync.dma_start(out_ap[:], prob[:])

    split_multi_waits(nc)
    return nc


def _prep_weights(inputs):
    """Host-side weight marshalling (layouts only, plus bias folding)."""
    g = {k: np.asarray(v, dtype=np.float32) for k, v in inputs.items()
         if k not in ("context", "options", "context_lens", "option_lens")}
    wm = {}
    for d, sfx in (("f", "_f"), ("b", "_b")):
        wm[f"wihT_{d}"] = np.ascontiguousarray(g["W_ih" + sfx].T)        # [300, 384]
        wm[f"whhT_{d}"] = np.ascontiguousarray(
            g["W_hh" + sfx].T.astype(ml_dtypes.bfloat16))                # [128, 384]
        bih, bhh = g["b_ih" + sfx], g["b_hh" + sfx]
        b3 = np.stack([bih[0:128] + bhh[0:128],
                       bih[128:256] + bhh[128:256],
                       bih[256:384]], axis=1)                            # [128, 3]
        wm[f"bias3_{d}"] = np.ascontiguousarray(b3)
        wm[f"bhhn_{d}"] = np.ascontiguousarray(bhh[256:384][:, None])    # [128, 1]
        wm[f"awihT_{d}"] = np.ascontiguousarray(
            g["aW_ih" + sfx].T.astype(ml_dtypes.bfloat16))               # [1024, 384]
        wm[f"awhhT_{d}"] = np.ascontiguousarray(
            g["aW_hh" + sfx].T.astype(ml_dtypes.bfloat16))
        abih, abhh = g["ab_ih" + sfx], g["ab_hh" + sfx]
        ab3 = np.stack([abih[0:128] + abhh[0:128],
                        abih[128:256] + abhh[128:256],
                        abih[256:384]], axis=1)
        wm[f"abias3_{d}"] = np.ascontiguousarray(ab3)
        wm[f"abhhn_{d}"] = np.ascontiguousarray(abhh[256:384][:, None])
    wm["wkT"] = np.ascontiguousarray(g["Wk"].T.astype(ml_dtypes.bfloat16))
    wm["wqT"] = np.ascontiguousarray(g["Wq"].T.astype(ml_dtypes.bfloat16))
    wm["wemat"] = np.ascontiguousarray(g["We"])
    wm["vvec"] = np.ascontiguousarray(g["v"][:, None])
    wm["wsimT"] = np.ascontiguousarray(
        g["Wsim"][0][:, None].astype(ml_dtypes.bfloat16))                # [512, 1]
    return wm


def kernel(**inputs):
    if "nc" not in _BUILT:
        _BUILT["nc"] = _build()
    nc = _BUILT["nc"]
    context = np.asarray(inputs["context"], dtype=np.float32)   # [32, 100, 300]
    options = np.asarray(inputs["options"], dtype=np.float32)   # [32, 5, 50, 300]
    wm = _prep_weights(inputs)
    B = context.shape[0]
    in_maps = []
    for c in range(NCORES):
        bs = slice(c * B4, (c + 1) * B4)
        ctx_sh = context[bs]                       # [4, 100, 300]
        opt_sh = options[bs]                       # [4, 5, 50, 300]
        m = dict(wm)
        # (e, (t, b)) and (e, (t, k*4+b))
        m["ctxT"] = np.ascontiguousarray(ctx_sh.transpose(2, 1, 0).reshape(E, NCTX))
        m["optT"] = np.ascontiguousarray(
            opt_sh.transpose(3, 2, 1, 0).reshape(E, NOPT))
        in_maps.append(m)
    res = run_bass_kernel_spmd(nc, in_maps, list(range(NCORES)))
    out = np.concatenate([res.results[c]["out"] for c in range(NCORES)], axis=0)
    if DEBUG:
        kernel.debug = [res.results[c] for c in range(NCORES)]
    return out.astype(np.float32)

